# revision 48
# baseline (speedup 1.0000x reference)
"""Attention-pooling Trainium2 kernel, v3.

Structure: the proven baseline pipeline (1024-wide psum score chunks,
exp on ACT with accum rowsums / DVE reduce mix, 4-strip col-tiled colsum
matmuls), with two algebraic offloads:

  * Q/K projections (2% of FLOPs) run on the host in fp32; the device
    receives q^T / k^T as bf16 in the [e_lo, eo, s] layout the score
    matmuls want.
  * mean_s(dist @ V) = (colsum(dist)/S) @ V: the device only computes
    w = colsum(dist) (already produced by the colsum matmuls) and ships
    it; the host finishes with (w@x)@Wv/S + bv (0.05% of the FLOPs).
    This removes the V projection, the w transposes and the final
    matvec from the device entirely.

Per-core: 8 cores = 4 batches x 2 query-row halves; each core computes
its own [2048, 4096] score block, exp, and recip-weighted column sums.
"""

import numpy as np

import concourse.bass as bass  # noqa: F401
import concourse.mybir as mybir
import concourse.tile as tile
from concourse import bacc

B, S, E = 4, 4096, 256
HALF = S // 2          # query rows per core
NORM = 16.0            # sqrt(E)
P = 128
N_CORES = 8
QTILES = HALF // P     # 16
F32 = mybir.dt.float32
BF16 = mybir.dt.bfloat16

EXPW = 1024            # exp() chunk width (PSUM tile width)
NEXP = S // EXPW       # 4 chunks per q-tile


def _emit(ctx, tc):
    nc = tc.nc

    qt_d = nc.dram_tensor("qt16", [2, P, HALF], BF16, kind="ExternalInput")
    kt_d = nc.dram_tensor("kt16", [2, P, S], BF16, kind="ExternalInput")
    w_d = nc.dram_tensor("w", [4, 1024], F32, kind="ExternalOutput")

    const = ctx.enter_context(tc.tile_pool(name="const", bufs=1))
    epool = ctx.enter_context(tc.tile_pool(name="epool", bufs=4))
    rsp = ctx.enter_context(tc.tile_pool(name="rsp", bufs=3))
    pp = ctx.enter_context(tc.tile_pool(name="pp", bufs=3, space="PSUM"))
    wp = ctx.enter_context(tc.tile_pool(name="wp", bufs=1, space="PSUM"))

    # ---- input DMAs: small first chunks so the first score tile is
    # unblocked early, then big chunks; spread over both hw rings.
    qt_sb = [const.tile([P, HALF], BF16, name=f"qt{eo}") for eo in range(2)]
    kt_sb = [const.tile([P, S], BF16, name=f"kt{eo}") for eo in range(2)]
    # first q-tile needs only qt cols 0:128 and kt cols 0:512 -- load those
    # first so the score pipeline starts as early as possible.
    bounds = [(0, 512), (512, 512), (1024, 1024), (2048, 2048)]
    for eo in range(2):
        (nc.sync if eo else nc.scalar).dma_start(
            out=qt_sb[eo][:, 0:128], in_=qt_d[eo, :, 0:128]
        )
    for ci, (c0, cw) in enumerate(bounds):
        for eo in range(2):
            eng = nc.sync if (ci + eo) % 2 else nc.scalar
            eng.dma_start(
                out=kt_sb[eo][:, c0 : c0 + cw], in_=kt_d[eo, :, c0 : c0 + cw]
            )
    for eo in range(2):
        nc.scalar.dma_start(out=qt_sb[eo][:, 128:1024], in_=qt_d[eo, :, 128:1024])
        nc.sync.dma_start(out=qt_sb[eo][:, 1024:2048], in_=qt_d[eo, :, 1024:2048])

    # ---- w accumulators (memset so full-width export copies are defined)
    w_ps = [wp.tile([P, 512], F32, tag=f"w{i}", name=f"w_ps{i}") for i in range(2)]
    for i in range(2):
        nc.vector.memset(w_ps[i], 0.0)

    def w_slot(m):
        return w_ps[m // 4], 32 * (m % 4)

    def emit_scores(qi):
        # all rowsums on DVE (one full-row reduce per tile): keeps ACT to
        # pure exp work -- no accumulator reads on the pacing engine.
        act_rowsum = False
        Es = epool.tile([P, S], BF16, tag="E", name=f"E{qi}")
        rs = None
        if act_rowsum:
            rs = rsp.tile([P, NEXP], F32, tag="rs", name=f"rs{qi}")
        for tc4 in range(NEXP):
            ps = pp.tile([P, EXPW], F32, tag="ps", name=f"ps_s{qi}_{tc4}")
            for eo in range(2):
                lhsT = qt_sb[eo][:, qi * P : (qi + 1) * P]
                for h in range(EXPW // 512):
                    t0 = tc4 * EXPW + h * 512
                    nc.tensor.matmul(
                        ps[:, h * 512 : (h + 1) * 512],
                        lhsT,
                        kt_sb[eo][:, t0 : t0 + 512],
                        start=(eo == 0),
                        stop=(eo == 1),
                    )
            nc.scalar.activation(
                out=Es[:, tc4 * EXPW : (tc4 + 1) * EXPW],
                in_=ps,
                func=mybir.ActivationFunctionType.Exp,
                scale=1.0 / NORM,
                accum_out=rs[:, tc4 : tc4 + 1] if act_rowsum else None,
            )
        rsum = rsp.tile([P, 1], F32, tag="rsum", name=f"rsum{qi}")
        if act_rowsum:
            nc.vector.reduce_sum(out=rsum, in_=rs, axis=mybir.AxisListType.X)
        else:
            nc.vector.reduce_sum(out=rsum, in_=Es, axis=mybir.AxisListType.X)
        recf = rsp.tile([P, 1], F32, tag="recf", name=f"recf{qi}")
        nc.vector.reciprocal(out=recf, in_=rsum)
        recb = rsp.tile([P, 1], BF16, tag="recb", name=f"recb{qi}")
        nc.vector.tensor_copy(out=recb, in_=recf)
        return Es, recb

    def emit_colsum(qi, Es, recb):
        for m in range(8):
            wt, strip = w_slot(m)
            nc.tensor.matmul(
                wt[strip : strip + 1, :],
                recb,
                Es[:, m * 512 : (m + 1) * 512],
                start=(qi == 0),
                stop=(qi == QTILES - 1),
                tile_position=(0, strip),
            )

    # software-pipeline colsum by 3 q-tiles (recip chain latency)
    pending = {}
    for qi in range(QTILES):
        pending[qi] = emit_scores(qi)
        if qi - 3 in pending:
            emit_colsum(qi - 3, *pending.pop(qi - 3))
    for qi in sorted(pending):
        emit_colsum(qi, *pending.pop(qi))

    # ---- export w: full-width psum->sbuf copies, then one strided DMA
    w_sb = const.tile([P, 1024], F32, name="w_sb")
    nc.vector.tensor_copy(out=w_sb[:, 0:512], in_=w_ps[0])
    nc.vector.tensor_copy(out=w_sb[:, 512:1024], in_=w_ps[1])
    out_src = bass.AP(
        tensor=w_sb.tensor,
        offset=w_sb.offset,
        ap=[[w_sb.ap[0][0] * 32, 4], [1, 1024]],
    )
    nc.sync.dma_start(out=w_d[:, :], in_=out_src)


_NC_CACHE = None


def _build_nc():
    global _NC_CACHE
    if _NC_CACHE is None:
        from contextlib import ExitStack

        nc = bacc.Bacc("TRN2", target_bir_lowering=False, debug=False)
        with tile.TileContext(nc) as tc, ExitStack() as ctx:
            _emit(ctx, tc)
        nc.compile()
        _NC_CACHE = nc
    return _NC_CACHE


def _in_maps(inputs):
    import ml_dtypes

    bf16 = ml_dtypes.bfloat16
    x = np.asarray(inputs["x"], dtype=np.float32)
    Wq = np.asarray(inputs["Wq"], dtype=np.float32)
    Wk = np.asarray(inputs["Wk"], dtype=np.float32)
    bq = np.asarray(inputs["bq"], dtype=np.float32)
    bk = np.asarray(inputs["bk"], dtype=np.float32)
    maps = []
    for c in range(N_CORES):
        b, h = divmod(c, 2)
        q = x[b, h * HALF : (h + 1) * HALF] @ Wq + bq        # [HALF, E]
        k = x[b] @ Wk + bk                                   # [S, E]
        qt16 = np.ascontiguousarray(q.T.reshape(2, P, HALF)).astype(bf16)
        kt16 = np.ascontiguousarray(k.T.reshape(2, P, S)).astype(bf16)
        maps.append({"qt16": qt16, "kt16": kt16})
    return maps


def _combine(results, inputs):
    x = np.asarray(inputs["x"], dtype=np.float64)
    Wv = np.asarray(inputs["Wv"], dtype=np.float64)
    bv = np.asarray(inputs["bv"], dtype=np.float64)
    out = np.empty((B, 1, E), dtype=np.float32)

    def _unpack(arr):
        # arr[k, 512*i + j] = w[2048*i + 512*k + j]
        return arr.reshape(4, 2, 512).transpose(1, 0, 2).reshape(S)

    for b in range(B):
        w = _unpack(
            results[2 * b]["w"].astype(np.float64)
            + results[2 * b + 1]["w"].astype(np.float64)
        )
        u = w @ x[b]
        out[b, 0] = ((u / S) @ Wv + bv).astype(np.float32)
    return out


def kernel(**inputs):
    from concourse.bass_utils import run_bass_kernel_spmd

    nc = _build_nc()
    res = run_bass_kernel_spmd(nc, _in_maps(inputs), core_ids=list(range(N_CORES)))
    return _combine(res.results, inputs)


# revision 49
# speedup vs baseline: 1.0518x; 1.0518x over previous
"""Attention-pooling Trainium2 kernel, v3.

Structure: the proven baseline pipeline (1024-wide psum score chunks,
exp on ACT with accum rowsums / DVE reduce mix, 4-strip col-tiled colsum
matmuls), with two algebraic offloads:

  * Q/K projections (2% of FLOPs) run on the host in fp32; the device
    receives q^T / k^T as bf16 in the [e_lo, eo, s] layout the score
    matmuls want.
  * mean_s(dist @ V) = (colsum(dist)/S) @ V: the device only computes
    w = colsum(dist) (already produced by the colsum matmuls) and ships
    it; the host finishes with (w@x)@Wv/S + bv (0.05% of the FLOPs).
    This removes the V projection, the w transposes and the final
    matvec from the device entirely.

Per-core: 8 cores = 4 batches x 2 query-row halves; each core computes
its own [2048, 4096] score block, exp, and recip-weighted column sums.
"""

import numpy as np

import concourse.bass as bass  # noqa: F401
import concourse.mybir as mybir
import concourse.tile as tile
from concourse import bacc

B, S, E = 4, 4096, 256
HALF = S // 2          # query rows per core
NORM = 16.0            # sqrt(E)
P = 128
N_CORES = 8
QTILES = HALF // P     # 16
F32 = mybir.dt.float32
BF16 = mybir.dt.bfloat16

EXPW = 1024            # exp() chunk width (PSUM tile width)
NEXP = S // EXPW       # 4 chunks per q-tile


def _emit(ctx, tc):
    nc = tc.nc

    qt_d = nc.dram_tensor("qt16", [2, P, HALF], BF16, kind="ExternalInput")
    kt_d = nc.dram_tensor("kt16", [2, P, S], BF16, kind="ExternalInput")
    w_d = nc.dram_tensor("w", [4, 1024], F32, kind="ExternalOutput")

    const = ctx.enter_context(tc.tile_pool(name="const", bufs=1))
    epool = ctx.enter_context(tc.tile_pool(name="epool", bufs=4))
    rsp = ctx.enter_context(tc.tile_pool(name="rsp", bufs=3))
    pp = ctx.enter_context(tc.tile_pool(name="pp", bufs=3, space="PSUM"))
    wp = ctx.enter_context(tc.tile_pool(name="wp", bufs=1, space="PSUM"))

    # ---- input DMAs: small first chunks so the first score tile is
    # unblocked early, then big chunks; spread over both hw rings.
    qt_sb = [const.tile([P, HALF], BF16, name=f"qt{eo}") for eo in range(2)]
    kt_sb = [const.tile([P, S], BF16, name=f"kt{eo}") for eo in range(2)]
    bounds = [(0, 512), (512, 512), (1024, 1024), (2048, 2048)]
    for eo in range(2):
        nc.sync.dma_start(out=qt_sb[eo][:, 0:1024], in_=qt_d[eo, :, 0:1024])
    for ci, (c0, cw) in enumerate(bounds):
        for eo in range(2):
            eng = nc.sync if (ci + eo) % 2 else nc.scalar
            eng.dma_start(
                out=kt_sb[eo][:, c0 : c0 + cw], in_=kt_d[eo, :, c0 : c0 + cw]
            )
    for eo in range(2):
        nc.scalar.dma_start(out=qt_sb[eo][:, 1024:2048], in_=qt_d[eo, :, 1024:2048])

    # ---- w accumulators (memset so full-width export copies are defined)
    w_ps = [wp.tile([P, 512], F32, tag=f"w{i}", name=f"w_ps{i}") for i in range(2)]
    for i in range(2):
        nc.vector.memset(w_ps[i], 0.0)

    def w_slot(m):
        return w_ps[m // 4], 32 * (m % 4)

    def emit_scores(qi):
        # rowsum load-balancing (as in the proven baseline): even/late
        # q-tiles use ACT's per-chunk accumulator, the rest one DVE reduce.
        act_rowsum = qi % 2 == 0 or qi >= 14
        Es = epool.tile([P, S], BF16, tag="E", name=f"E{qi}")
        rs = None
        if act_rowsum:
            rs = rsp.tile([P, NEXP], F32, tag="rs", name=f"rs{qi}")
        for tc4 in range(NEXP):
            ps = pp.tile([P, EXPW], F32, tag="ps", name=f"ps_s{qi}_{tc4}")
            for eo in range(2):
                lhsT = qt_sb[eo][:, qi * P : (qi + 1) * P]
                for h in range(EXPW // 512):
                    t0 = tc4 * EXPW + h * 512
                    nc.tensor.matmul(
                        ps[:, h * 512 : (h + 1) * 512],
                        lhsT,
                        kt_sb[eo][:, t0 : t0 + 512],
                        start=(eo == 0),
                        stop=(eo == 1),
                    )
            nc.scalar.activation(
                out=Es[:, tc4 * EXPW : (tc4 + 1) * EXPW],
                in_=ps,
                func=mybir.ActivationFunctionType.Exp,
                scale=1.0 / NORM,
                accum_out=rs[:, tc4 : tc4 + 1] if act_rowsum else None,
            )
        rsum = rsp.tile([P, 1], F32, tag="rsum", name=f"rsum{qi}")
        if act_rowsum:
            nc.vector.reduce_sum(out=rsum, in_=rs, axis=mybir.AxisListType.X)
        else:
            nc.vector.reduce_sum(out=rsum, in_=Es, axis=mybir.AxisListType.X)
        recf = rsp.tile([P, 1], F32, tag="recf", name=f"recf{qi}")
        nc.vector.reciprocal(out=recf, in_=rsum)
        recb = rsp.tile([P, 1], BF16, tag="recb", name=f"recb{qi}")
        nc.vector.tensor_copy(out=recb, in_=recf)
        return Es, recb

    def emit_colsum(qi, Es, recb):
        for m in range(8):
            wt, strip = w_slot(m)
            nc.tensor.matmul(
                wt[strip : strip + 1, :],
                recb,
                Es[:, m * 512 : (m + 1) * 512],
                start=(qi == 0),
                stop=(qi == QTILES - 1),
                tile_position=(0, strip),
            )

    # software-pipeline colsum by 3 q-tiles (recip chain latency)
    pending = {}
    for qi in range(QTILES):
        pending[qi] = emit_scores(qi)
        if qi - 3 in pending:
            emit_colsum(qi - 3, *pending.pop(qi - 3))
    for qi in sorted(pending):
        emit_colsum(qi, *pending.pop(qi))

    # ---- export w: full-width psum->sbuf copies, then one strided DMA
    w_sb = const.tile([P, 1024], F32, name="w_sb")
    nc.vector.tensor_copy(out=w_sb[:, 0:512], in_=w_ps[0])
    nc.scalar.copy(out=w_sb[:, 512:1024], in_=w_ps[1])
    out_src = bass.AP(
        tensor=w_sb.tensor,
        offset=w_sb.offset,
        ap=[[w_sb.ap[0][0] * 32, 4], [1, 1024]],
    )
    nc.sync.dma_start(out=w_d[:, :], in_=out_src)


_NC_CACHE = None


def _build_nc():
    global _NC_CACHE
    if _NC_CACHE is None:
        from contextlib import ExitStack

        nc = bacc.Bacc("TRN2", target_bir_lowering=False, debug=False)
        with tile.TileContext(nc) as tc, ExitStack() as ctx:
            _emit(ctx, tc)
        nc.compile()
        _NC_CACHE = nc
    return _NC_CACHE


def _in_maps(inputs):
    import ml_dtypes

    bf16 = ml_dtypes.bfloat16
    x = np.asarray(inputs["x"], dtype=np.float32)
    Wq = np.asarray(inputs["Wq"], dtype=np.float32)
    Wk = np.asarray(inputs["Wk"], dtype=np.float32)
    bq = np.asarray(inputs["bq"], dtype=np.float32)
    bk = np.asarray(inputs["bk"], dtype=np.float32)
    maps = []
    for c in range(N_CORES):
        b, h = divmod(c, 2)
        q = x[b, h * HALF : (h + 1) * HALF] @ Wq + bq        # [HALF, E]
        k = x[b] @ Wk + bk                                   # [S, E]
        qt16 = np.ascontiguousarray(q.T.reshape(2, P, HALF)).astype(bf16)
        kt16 = np.ascontiguousarray(k.T.reshape(2, P, S)).astype(bf16)
        maps.append({"qt16": qt16, "kt16": kt16})
    return maps


def _combine(results, inputs):
    x = np.asarray(inputs["x"], dtype=np.float64)
    Wv = np.asarray(inputs["Wv"], dtype=np.float64)
    bv = np.asarray(inputs["bv"], dtype=np.float64)
    out = np.empty((B, 1, E), dtype=np.float32)

    def _unpack(arr):
        # arr[k, 512*i + j] = w[2048*i + 512*k + j]
        return arr.reshape(4, 2, 512).transpose(1, 0, 2).reshape(S)

    for b in range(B):
        w = _unpack(
            results[2 * b]["w"].astype(np.float64)
            + results[2 * b + 1]["w"].astype(np.float64)
        )
        u = w @ x[b]
        out[b, 0] = ((u / S) @ Wv + bv).astype(np.float32)
    return out


def kernel(**inputs):
    from concourse.bass_utils import run_bass_kernel_spmd

    nc = _build_nc()
    res = run_bass_kernel_spmd(nc, _in_maps(inputs), core_ids=list(range(N_CORES)))
    return _combine(res.results, inputs)


# revision 50
# speedup vs baseline: 1.0570x; 1.0050x over previous
"""Attention-pooling Trainium2 kernel, v3.

Structure: the proven baseline pipeline (1024-wide psum score chunks,
exp on ACT with accum rowsums / DVE reduce mix, 4-strip col-tiled colsum
matmuls), with two algebraic offloads:

  * Q/K projections (2% of FLOPs) run on the host in fp32; the device
    receives q^T / k^T as bf16 in the [e_lo, eo, s] layout the score
    matmuls want.
  * mean_s(dist @ V) = (colsum(dist)/S) @ V: the device only computes
    w = colsum(dist) (already produced by the colsum matmuls) and ships
    it; the host finishes with (w@x)@Wv/S + bv (0.05% of the FLOPs).
    This removes the V projection, the w transposes and the final
    matvec from the device entirely.

Per-core: 8 cores = 4 batches x 2 query-row halves; each core computes
its own [2048, 4096] score block, exp, and recip-weighted column sums.
"""

import numpy as np

import concourse.bass as bass  # noqa: F401
import concourse.mybir as mybir
import concourse.tile as tile
from concourse import bacc

B, S, E = 4, 4096, 256
HALF = S // 2          # query rows per core
NORM = 16.0            # sqrt(E)
P = 128
N_CORES = 8
QTILES = HALF // P     # 16
F32 = mybir.dt.float32
BF16 = mybir.dt.bfloat16

EXPW = 1024            # exp() chunk width (PSUM tile width)
NEXP = S // EXPW       # 4 chunks per q-tile


def _emit(ctx, tc):
    nc = tc.nc

    qt_d = nc.dram_tensor("qt16", [2, P, HALF], BF16, kind="ExternalInput")
    kt_d = nc.dram_tensor("kt16", [2, P, S], BF16, kind="ExternalInput")
    w_d = nc.dram_tensor("w", [4, 1024], F32, kind="ExternalOutput")

    const = ctx.enter_context(tc.tile_pool(name="const", bufs=1))
    epool = ctx.enter_context(tc.tile_pool(name="epool", bufs=4))
    rsp = ctx.enter_context(tc.tile_pool(name="rsp", bufs=3))
    pp = ctx.enter_context(tc.tile_pool(name="pp", bufs=3, space="PSUM"))
    wp = ctx.enter_context(tc.tile_pool(name="wp", bufs=1, space="PSUM"))

    # ---- input DMAs: small first chunks so the first score tile is
    # unblocked early, then big chunks; spread over both hw rings.
    qt_sb = [const.tile([P, HALF], BF16, name=f"qt{eo}") for eo in range(2)]
    kt_sb = [const.tile([P, S], BF16, name=f"kt{eo}") for eo in range(2)]
    # first q-tile needs only qt cols 0:128 and kt cols 0:512 -- load those
    # first so the score pipeline starts as early as possible.
    bounds = [(0, 512), (512, 512), (1024, 1024), (2048, 2048)]
    for eo in range(2):
        (nc.sync if eo else nc.scalar).dma_start(
            out=qt_sb[eo][:, 0:128], in_=qt_d[eo, :, 0:128]
        )
    for ci, (c0, cw) in enumerate(bounds):
        for eo in range(2):
            eng = nc.sync if (ci + eo) % 2 else nc.scalar
            eng.dma_start(
                out=kt_sb[eo][:, c0 : c0 + cw], in_=kt_d[eo, :, c0 : c0 + cw]
            )
    for eo in range(2):
        nc.scalar.dma_start(out=qt_sb[eo][:, 128:1024], in_=qt_d[eo, :, 128:1024])
        nc.sync.dma_start(out=qt_sb[eo][:, 1024:2048], in_=qt_d[eo, :, 1024:2048])

    # ---- w accumulators (memset so full-width export copies are defined)
    w_ps = [wp.tile([P, 512], F32, tag=f"w{i}", name=f"w_ps{i}") for i in range(2)]
    for i in range(2):
        nc.vector.memset(w_ps[i], 0.0)

    def w_slot(m):
        return w_ps[m // 4], 32 * (m % 4)

    def emit_scores(qi):
        # rowsum load-balancing (as in the proven baseline): even/late
        # q-tiles use ACT's per-chunk accumulator, the rest one DVE reduce.
        act_rowsum = qi % 2 == 0 or qi >= 14
        Es = epool.tile([P, S], BF16, tag="E", name=f"E{qi}")
        rs = None
        if act_rowsum:
            rs = rsp.tile([P, NEXP], F32, tag="rs", name=f"rs{qi}")
        for tc4 in range(NEXP):
            ps = pp.tile([P, EXPW], F32, tag="ps", name=f"ps_s{qi}_{tc4}")
            for eo in range(2):
                lhsT = qt_sb[eo][:, qi * P : (qi + 1) * P]
                for h in range(EXPW // 512):
                    t0 = tc4 * EXPW + h * 512
                    nc.tensor.matmul(
                        ps[:, h * 512 : (h + 1) * 512],
                        lhsT,
                        kt_sb[eo][:, t0 : t0 + 512],
                        start=(eo == 0),
                        stop=(eo == 1),
                    )
            nc.scalar.activation(
                out=Es[:, tc4 * EXPW : (tc4 + 1) * EXPW],
                in_=ps,
                func=mybir.ActivationFunctionType.Exp,
                scale=1.0 / NORM,
                accum_out=rs[:, tc4 : tc4 + 1] if act_rowsum else None,
            )
        rsum = rsp.tile([P, 1], F32, tag="rsum", name=f"rsum{qi}")
        if act_rowsum:
            nc.vector.reduce_sum(out=rsum, in_=rs, axis=mybir.AxisListType.X)
        else:
            nc.vector.reduce_sum(out=rsum, in_=Es, axis=mybir.AxisListType.X)
        recf = rsp.tile([P, 1], F32, tag="recf", name=f"recf{qi}")
        nc.vector.reciprocal(out=recf, in_=rsum)
        recb = rsp.tile([P, 1], BF16, tag="recb", name=f"recb{qi}")
        nc.vector.tensor_copy(out=recb, in_=recf)
        return Es, recb

    def emit_colsum(qi, Es, recb):
        for m in range(8):
            wt, strip = w_slot(m)
            nc.tensor.matmul(
                wt[strip : strip + 1, :],
                recb,
                Es[:, m * 512 : (m + 1) * 512],
                start=(qi == 0),
                stop=(qi == QTILES - 1),
                tile_position=(0, strip),
            )

    # software-pipeline colsum by 3 q-tiles (recip chain latency)
    pending = {}
    for qi in range(QTILES):
        pending[qi] = emit_scores(qi)
        if qi - 3 in pending:
            emit_colsum(qi - 3, *pending.pop(qi - 3))
    for qi in sorted(pending):
        emit_colsum(qi, *pending.pop(qi))

    # ---- export w: full-width psum->sbuf copies, then one strided DMA
    w_sb = const.tile([P, 1024], F32, name="w_sb")
    nc.vector.tensor_copy(out=w_sb[:, 0:512], in_=w_ps[0])
    nc.scalar.copy(out=w_sb[:, 512:1024], in_=w_ps[1])
    out_src = bass.AP(
        tensor=w_sb.tensor,
        offset=w_sb.offset,
        ap=[[w_sb.ap[0][0] * 32, 4], [1, 1024]],
    )
    nc.sync.dma_start(out=w_d[:, :], in_=out_src)


_NC_CACHE = None


def _build_nc():
    global _NC_CACHE
    if _NC_CACHE is None:
        from contextlib import ExitStack

        nc = bacc.Bacc("TRN2", target_bir_lowering=False, debug=False)
        with tile.TileContext(nc) as tc, ExitStack() as ctx:
            _emit(ctx, tc)
        nc.compile()
        _NC_CACHE = nc
    return _NC_CACHE


def _in_maps(inputs):
    import ml_dtypes

    bf16 = ml_dtypes.bfloat16
    x = np.asarray(inputs["x"], dtype=np.float32)
    Wq = np.asarray(inputs["Wq"], dtype=np.float32)
    Wk = np.asarray(inputs["Wk"], dtype=np.float32)
    bq = np.asarray(inputs["bq"], dtype=np.float32)
    bk = np.asarray(inputs["bk"], dtype=np.float32)
    maps = []
    for c in range(N_CORES):
        b, h = divmod(c, 2)
        q = x[b, h * HALF : (h + 1) * HALF] @ Wq + bq        # [HALF, E]
        k = x[b] @ Wk + bk                                   # [S, E]
        qt16 = np.ascontiguousarray(q.T.reshape(2, P, HALF)).astype(bf16)
        kt16 = np.ascontiguousarray(k.T.reshape(2, P, S)).astype(bf16)
        maps.append({"qt16": qt16, "kt16": kt16})
    return maps


def _combine(results, inputs):
    x = np.asarray(inputs["x"], dtype=np.float64)
    Wv = np.asarray(inputs["Wv"], dtype=np.float64)
    bv = np.asarray(inputs["bv"], dtype=np.float64)
    out = np.empty((B, 1, E), dtype=np.float32)

    def _unpack(arr):
        # arr[k, 512*i + j] = w[2048*i + 512*k + j]
        return arr.reshape(4, 2, 512).transpose(1, 0, 2).reshape(S)

    for b in range(B):
        w = _unpack(
            results[2 * b]["w"].astype(np.float64)
            + results[2 * b + 1]["w"].astype(np.float64)
        )
        u = w @ x[b]
        out[b, 0] = ((u / S) @ Wv + bv).astype(np.float32)
    return out


def kernel(**inputs):
    from concourse.bass_utils import run_bass_kernel_spmd

    nc = _build_nc()
    res = run_bass_kernel_spmd(nc, _in_maps(inputs), core_ids=list(range(N_CORES)))
    return _combine(res.results, inputs)


# revision 51
# speedup vs baseline: 1.0609x; 1.0036x over previous
"""Attention-pooling Trainium2 kernel, v3.

Structure: the proven baseline pipeline (1024-wide psum score chunks,
exp on ACT with accum rowsums / DVE reduce mix, 4-strip col-tiled colsum
matmuls), with two algebraic offloads:

  * Q/K projections (2% of FLOPs) run on the host in fp32; the device
    receives q^T / k^T as bf16 in the [e_lo, eo, s] layout the score
    matmuls want.
  * mean_s(dist @ V) = (colsum(dist)/S) @ V: the device only computes
    w = colsum(dist) (already produced by the colsum matmuls) and ships
    it; the host finishes with (w@x)@Wv/S + bv (0.05% of the FLOPs).
    This removes the V projection, the w transposes and the final
    matvec from the device entirely.

Per-core: 8 cores = 4 batches x 2 query-row halves; each core computes
its own [2048, 4096] score block, exp, and recip-weighted column sums.
"""

import numpy as np

import concourse.bass as bass  # noqa: F401
import concourse.mybir as mybir
import concourse.tile as tile
from concourse import bacc

B, S, E = 4, 4096, 256
HALF = S // 2          # query rows per core
NORM = 16.0            # sqrt(E)
P = 128
N_CORES = 8
QTILES = HALF // P     # 16
F32 = mybir.dt.float32
BF16 = mybir.dt.bfloat16

EXPW = 1024            # exp() chunk width (PSUM tile width)
NEXP = S // EXPW       # 4 chunks per q-tile


def _emit(ctx, tc):
    nc = tc.nc

    qt_d = nc.dram_tensor("qt16", [2, P, HALF], BF16, kind="ExternalInput")
    kt_d = nc.dram_tensor("kt16", [2, P, S], BF16, kind="ExternalInput")
    w_d = nc.dram_tensor("w", [4, 1024], F32, kind="ExternalOutput")

    const = ctx.enter_context(tc.tile_pool(name="const", bufs=1))
    epool = ctx.enter_context(tc.tile_pool(name="epool", bufs=4))
    rsp = ctx.enter_context(tc.tile_pool(name="rsp", bufs=3))
    pp = ctx.enter_context(tc.tile_pool(name="pp", bufs=3, space="PSUM"))
    wp = ctx.enter_context(tc.tile_pool(name="wp", bufs=1, space="PSUM"))

    # ---- input DMAs: small first chunks so the first score tile is
    # unblocked early, then big chunks; spread over both hw rings.
    qt_sb = [const.tile([P, HALF], BF16, name=f"qt{eo}") for eo in range(2)]
    kt_sb = [const.tile([P, S], BF16, name=f"kt{eo}") for eo in range(2)]
    # first q-tile needs only qt cols 0:128 and kt cols 0:512 -- load those
    # first so the score pipeline starts as early as possible.
    bounds = [(0, 512), (512, 512), (1024, 1024), (2048, 2048)]
    for eo in range(2):
        (nc.sync if eo else nc.scalar).dma_start(
            out=qt_sb[eo][:, 0:128], in_=qt_d[eo, :, 0:128]
        )
    for ci, (c0, cw) in enumerate(bounds):
        for eo in range(2):
            eng = nc.sync if (ci + eo) % 2 else nc.scalar
            eng.dma_start(
                out=kt_sb[eo][:, c0 : c0 + cw], in_=kt_d[eo, :, c0 : c0 + cw]
            )
    for eo in range(2):
        nc.scalar.dma_start(out=qt_sb[eo][:, 128:1024], in_=qt_d[eo, :, 128:1024])
        nc.sync.dma_start(out=qt_sb[eo][:, 1024:2048], in_=qt_d[eo, :, 1024:2048])

    # ---- w accumulators (memset so full-width export copies are defined)
    w_ps = [wp.tile([P, 512], F32, tag=f"w{i}", name=f"w_ps{i}") for i in range(2)]
    for i in range(2):
        nc.vector.memset(w_ps[i], 0.0)

    def w_slot(m):
        return w_ps[m // 4], 32 * (m % 4)

    def emit_scores(qi):
        # rowsum load-balancing (as in the proven baseline): even/late
        # q-tiles use ACT's per-chunk accumulator, the rest one DVE reduce.
        act_rowsum = qi % 2 == 0 or qi >= 14
        Es = epool.tile([P, S], BF16, tag="E", name=f"E{qi}")
        rs = None
        if act_rowsum:
            rs = rsp.tile([P, NEXP], F32, tag="rs", name=f"rs{qi}")
        for tc4 in range(NEXP):
            ps = pp.tile([P, EXPW], F32, tag="ps", name=f"ps_s{qi}_{tc4}")
            for eo in range(2):
                lhsT = qt_sb[eo][:, qi * P : (qi + 1) * P]
                for h in range(EXPW // 512):
                    t0 = tc4 * EXPW + h * 512
                    nc.tensor.matmul(
                        ps[:, h * 512 : (h + 1) * 512],
                        lhsT,
                        kt_sb[eo][:, t0 : t0 + 512],
                        start=(eo == 0),
                        stop=(eo == 1),
                    )
            nc.scalar.activation(
                out=Es[:, tc4 * EXPW : (tc4 + 1) * EXPW],
                in_=ps,
                func=mybir.ActivationFunctionType.Exp,
                scale=1.0 / NORM,
                accum_out=rs[:, tc4 : tc4 + 1] if act_rowsum else None,
            )
        rsum = rsp.tile([P, 1], F32, tag="rsum", name=f"rsum{qi}")
        if act_rowsum:
            nc.vector.reduce_sum(out=rsum, in_=rs, axis=mybir.AxisListType.X)
        else:
            nc.vector.reduce_sum(out=rsum, in_=Es, axis=mybir.AxisListType.X)
        recf = rsp.tile([P, 1], F32, tag="recf", name=f"recf{qi}")
        nc.vector.reciprocal(out=recf, in_=rsum)
        recb = rsp.tile([P, 1], BF16, tag="recb", name=f"recb{qi}")
        nc.vector.tensor_copy(out=recb, in_=recf)
        return Es, recb

    def emit_colsum(qi, Es, recb):
        for m in range(8):
            wt, strip = w_slot(m)
            nc.tensor.matmul(
                wt[strip : strip + 1, :],
                recb,
                Es[:, m * 512 : (m + 1) * 512],
                start=(qi == 0),
                stop=(qi == QTILES - 1),
                tile_position=(0, strip),
            )

    # software-pipeline colsum by 3 q-tiles (recip chain latency)
    pending = {}
    for qi in range(QTILES):
        pending[qi] = emit_scores(qi)
        if qi - 3 in pending:
            emit_colsum(qi - 3, *pending.pop(qi - 3))
    for qi in sorted(pending):
        emit_colsum(qi, *pending.pop(qi))

    # ---- export w: full-width psum->sbuf copies, then one strided DMA
    w_sb = const.tile([P, 1024], F32, name="w_sb")
    # both copies on DVE: it is idle once the last colsum lands, while the
    # ACT queue is still draining its final exp chunks.
    nc.vector.tensor_copy(out=w_sb[:, 0:512], in_=w_ps[0])
    nc.vector.tensor_copy(out=w_sb[:, 512:1024], in_=w_ps[1])
    out_src = bass.AP(
        tensor=w_sb.tensor,
        offset=w_sb.offset,
        ap=[[w_sb.ap[0][0] * 32, 4], [1, 1024]],
    )
    nc.sync.dma_start(out=w_d[:, :], in_=out_src)


_NC_CACHE = None


def _build_nc():
    global _NC_CACHE
    if _NC_CACHE is None:
        from contextlib import ExitStack

        nc = bacc.Bacc("TRN2", target_bir_lowering=False, debug=False)
        with tile.TileContext(nc) as tc, ExitStack() as ctx:
            _emit(ctx, tc)
        nc.compile()
        _NC_CACHE = nc
    return _NC_CACHE


def _in_maps(inputs):
    import ml_dtypes

    bf16 = ml_dtypes.bfloat16
    x = np.asarray(inputs["x"], dtype=np.float32)
    Wq = np.asarray(inputs["Wq"], dtype=np.float32)
    Wk = np.asarray(inputs["Wk"], dtype=np.float32)
    bq = np.asarray(inputs["bq"], dtype=np.float32)
    bk = np.asarray(inputs["bk"], dtype=np.float32)
    maps = []
    for c in range(N_CORES):
        b, h = divmod(c, 2)
        q = x[b, h * HALF : (h + 1) * HALF] @ Wq + bq        # [HALF, E]
        k = x[b] @ Wk + bk                                   # [S, E]
        qt16 = np.ascontiguousarray(q.T.reshape(2, P, HALF)).astype(bf16)
        kt16 = np.ascontiguousarray(k.T.reshape(2, P, S)).astype(bf16)
        maps.append({"qt16": qt16, "kt16": kt16})
    return maps


def _combine(results, inputs):
    x = np.asarray(inputs["x"], dtype=np.float64)
    Wv = np.asarray(inputs["Wv"], dtype=np.float64)
    bv = np.asarray(inputs["bv"], dtype=np.float64)
    out = np.empty((B, 1, E), dtype=np.float32)

    def _unpack(arr):
        # arr[k, 512*i + j] = w[2048*i + 512*k + j]
        return arr.reshape(4, 2, 512).transpose(1, 0, 2).reshape(S)

    for b in range(B):
        w = _unpack(
            results[2 * b]["w"].astype(np.float64)
            + results[2 * b + 1]["w"].astype(np.float64)
        )
        u = w @ x[b]
        out[b, 0] = ((u / S) @ Wv + bv).astype(np.float32)
    return out


def kernel(**inputs):
    from concourse.bass_utils import run_bass_kernel_spmd

    nc = _build_nc()
    res = run_bass_kernel_spmd(nc, _in_maps(inputs), core_ids=list(range(N_CORES)))
    return _combine(res.results, inputs)
